# revision 7
# baseline (speedup 1.0000x reference)
"""Supervised-contrastive loss (balanced softmax variant) on 8 Trainium2 cores.

Transposed/class-sorted formulation. Rows are sorted by target class and
split 1024 per core; columns (all 8192 features + 1000 centers, merged and
class-sorted) are permuted per core so the core's "positive window" (all
columns whose class appears among its rows, <= 1280 of 9216) comes first.

Per j-tile (128 columns on partitions, 1024 rows on the free axis):
    PE : d[j, i] = A_j . f_i                    fp16 matmul, 2 x N=512
    ACT: E'[j, i] = exp(10*d + (ln a_j - 10))   a_j = 1/cls_count (the
         per-COLUMN balanced-softmax weight rides in the per-PARTITION
         activation bias -- no bias matmul at all)
    DVE: Acc[t%2] += E'          fp16 tensor_tensor add (2x_1p mode)
    DVE (window tiles only): MAcc += (tcol_j == trow_i) * E'

Final partition-reduction on the PE (ones-column matmuls) produces
S = colsum(Acc0+Acc1) and P = colsum(MAcc) as [1, 1024] rows; one 8 KB
DMA returns them. The host finishes in f64:
    S'_i  = S_i + (P_i - Eii)/n_i     n_i = bincount(targets)[t_i]
    mlp_i = 10*(f_i.M[t_i] - r2_i)/n_i - 10 - log(S'_i)
    loss  = -mean(mlp_i)
where Eii = exp(10*r2q_i - 10) removes the self column exactly (r2q is the
self dot in the same fp16 values the PE saw), and M[c] is the class sum of
fp16 features + center.
"""

import sys
from contextlib import ExitStack

import numpy as np
import ml_dtypes

sys.path.insert(0, "/opt/trn_rl_repo")

import concourse.bass as bass  # noqa: E402
import concourse.mybir as mybir  # noqa: E402
import concourse.tile as tile  # noqa: E402
from concourse import bacc  # noqa: E402
from concourse.bass_utils import run_bass_kernel_spmd  # noqa: E402

P = 128
BL = 1024          # rows per core
NT = 72            # j-tiles of 128 columns (9216 padded)
JP = NT * P
MW = 10            # masked-window j-tiles (1280 cols >= max window)
TEMP = 0.1
SHIFT = 10.0

PSUM_BUFS = 2
# fTq DMA chunk sizes in j-tiles: small first so the PE can start early
WCHUNKS = [1] * 4 + [2] * 4 + [4] * 3 + [6] * 8
assert sum(WCHUNKS) == NT
FCHUNK = 8         # featq DMA chunks

F16 = mybir.dt.float16
F32 = mybir.dt.float32
AF = mybir.ActivationFunctionType
ALU = mybir.AluOpType


def build_nc() -> bass.Bass:
    """One-core program; run SPMD on 8 cores with per-core inputs."""
    nc = bacc.Bacc(None)
    featq_d = nc.declare_dram_parameter("featq", [P, BL], F16, isOutput=False)
    fTq_d = nc.declare_dram_parameter("fTq", [P, NT * P], F16, isOutput=False)
    bias_d = nc.declare_dram_parameter("bias", [P, NT], F32, isOutput=False)
    tIrow_d = nc.declare_dram_parameter("tIrow", [1, BL], F16, isOutput=False)
    tcol_d = nc.declare_dram_parameter("tcol", [P, MW], F16, isOutput=False)
    out_d = nc.declare_dram_parameter("out", [1, 2 * BL], F32, isOutput=True)

    with tile.TileContext(nc) as tc, ExitStack() as ctx:
        const = ctx.enter_context(tc.tile_pool(name="const", bufs=1))
        epool = ctx.enter_context(tc.tile_pool(name="epool", bufs=3))
        mpool = ctx.enter_context(tc.tile_pool(name="mpool", bufs=2))
        psum = ctx.enter_context(
            tc.tile_pool(name="psum", bufs=PSUM_BUFS, space=bass.MemorySpace.PSUM)
        )
        rpsum = ctx.enter_context(
            tc.tile_pool(name="rpsum", bufs=1, space=bass.MemorySpace.PSUM)
        )

        # warm the ACT exp table while DMAs are in flight
        warm = const.tile([P, 1], F32)
        nc.vector.memset(warm[:], 0.0)
        nc.scalar.activation(warm[:], warm[:], AF.Exp, bias=warm[:], scale=1.0)

        # -- input DMAs; issue split between the Sync and GpSimd sequencers
        # (each dma_start costs ~600ns of serial issue time per sequencer)
        featq = const.tile([P, BL], F16)
        FS = BL // FCHUNK
        for c in range(FCHUNK):
            nc.sync.dma_start(featq[:, c * FS:(c + 1) * FS],
                              featq_d[:, c * FS:(c + 1) * FS])

        fTqs = []
        off = 0
        for ci, w in enumerate(WCHUNKS):
            ft = const.tile([P, w * P], F16, tag=f"fTq{ci}")
            eng = nc.sync if ci < 6 else nc.gpsimd
            eng.dma_start(ft[:], fTq_d[:, off * P:(off + w) * P])
            fTqs.append((off, w, ft))
            off += w

        bias = const.tile([P, NT], F32)
        nc.gpsimd.dma_start(bias[:], bias_d[:])
        tcol = const.tile([P, MW], F16)
        nc.gpsimd.dma_start(tcol[:], tcol_d[:])
        tIrow_r = const.tile([1, BL], F16)
        nc.gpsimd.dma_start(tIrow_r[:], tIrow_d[:])
        tIrow = const.tile([P, BL], F16)
        nc.gpsimd.partition_broadcast(tIrow[:], tIrow_r[:])

        acc0 = const.tile([P, BL], F16)
        nc.vector.memset(acc0[:], 0.0)
        acc1 = const.tile([P, BL], F16)
        nc.vector.memset(acc1[:], 0.0)
        macc = const.tile([P, BL], F16)
        nc.vector.memset(macc[:], 0.0)
        ones = const.tile([P, 1], F16)
        nc.vector.memset(ones[:], 1.0)
        accs = [acc0, acc1]

        outt = const.tile([1, 2 * BL], F32)
        pred = rpsum.tile([1, BL], F32, tag="pred")
        sred = rpsum.tile([1, BL], F32, tag="sred")

        def jtile(t):
            for off, w, ft in fTqs:
                if off <= t < off + w:
                    return ft[:, (t - off) * P:(t - off + 1) * P]
            raise AssertionError

        for t in range(NT):
            lhs = jtile(t)
            pt = psum.tile([P, BL], F32, tag="pt")
            for h in range(2):
                nc.tensor.matmul(
                    pt[:, h * 512:(h + 1) * 512], lhs,
                    featq[:, h * 512:(h + 1) * 512],
                    start=True, stop=True,
                )
            et = epool.tile([P, BL], F16, tag="et")
            nc.scalar.activation(
                et[:], pt[:], AF.Exp, bias=bias[:, t:t + 1], scale=SHIFT,
            )
            a = accs[t % 2]
            nc.vector.tensor_tensor(a[:], a[:], et[:], ALU.add)
            if t < MW:
                mt = mpool.tile([P, BL], F16, tag="mt")
                nc.vector.scalar_tensor_tensor(
                    out=mt[:], in0=tIrow[:], scalar=tcol[:, t:t + 1], in1=et[:],
                    op0=ALU.is_equal, op1=ALU.mult,
                )
                nc.vector.tensor_tensor(macc[:], macc[:], mt[:], ALU.add)
            if t == MW:
                # masked accumulator is final: reduce + stage its output early
                for h in range(2):
                    nc.tensor.matmul(pred[:, h * 512:(h + 1) * 512], ones[:],
                                     macc[:, h * 512:(h + 1) * 512],
                                     start=True, stop=True)
                nc.scalar.copy(outt[:, BL:2 * BL], pred[:])

        for h in range(2):
            nc.tensor.matmul(sred[:, h * 512:(h + 1) * 512], ones[:],
                             acc0[:, h * 512:(h + 1) * 512],
                             start=True, stop=False)
            nc.tensor.matmul(sred[:, h * 512:(h + 1) * 512], ones[:],
                             acc1[:, h * 512:(h + 1) * 512],
                             start=False, stop=True)
        nc.scalar.copy(outt[:, 0:BL], sred[:])
        nc.sync.dma_start(out_d[:], outt[:])

    nc.finalize()
    return nc


def prep_inputs(centers1, features, targets, n_cores):
    """Host-side sort/shard/layout prep. Returns per-core input maps and
    the per-core host epilogue data."""
    B, D = features.shape
    C = centers1.shape[0]
    J = B + C
    assert BL * n_cores == B and D == P and JP >= J

    feats16 = np.asarray(features, np.float32).astype(np.float16)
    cents16 = np.asarray(centers1, np.float32).astype(np.float16)
    targets = np.asarray(targets).astype(np.int64)

    n = np.bincount(targets, minlength=C).astype(np.int64)
    cc = n + 1

    order = np.argsort(targets, kind="stable")          # rows sorted by class
    # merged class-sorted columns: per class, feature rows then the center
    col_order = np.lexsort((
        np.concatenate([np.arange(B), np.full(C, B)]),
        np.concatenate([targets, np.arange(C)]),
    ))
    A16 = np.concatenate([feats16, cents16], axis=0)[col_order]   # [J, D]
    col_cls = np.concatenate([targets, np.arange(C)])[col_order]  # [J]
    a_col = 1.0 / cc[col_cls]

    # class sums for the numerator (f64 on the fp16 values)
    M = np.zeros((C, D))
    np.add.at(M, targets, feats16.astype(np.float64))
    M += cents16

    in_maps, host = [], []
    for k in range(n_cores):
        rids = order[k * BL:(k + 1) * BL]
        tcls = targets[rids]
        jlo = np.searchsorted(col_cls, tcls[0], "left")
        jhi = np.searchsorted(col_cls, tcls[-1], "right")
        assert jhi - jlo <= MW * P, f"core {k}: window {jhi - jlo} > {MW * P}"
        perm = np.concatenate([
            np.arange(jlo, jhi), np.arange(0, jlo), np.arange(jhi, J)
        ])

        Ap = np.zeros((JP, D), np.float16)
        Ap[:J] = A16[perm]
        up = np.full(JP, -1.0)
        up[:J] = col_cls[perm]
        biasv = np.full(JP, -200.0, np.float64)
        biasv[:J] = np.log(a_col[perm]) - SHIFT

        fq = feats16[rids]
        fTq = np.ascontiguousarray(
            Ap.reshape(NT, P, P).transpose(2, 0, 1).reshape(P, NT * P))
        featq = np.ascontiguousarray(fq.T)

        in_maps.append({
            "featq": featq,
            "fTq": fTq,
            "bias": np.ascontiguousarray(biasv.reshape(NT, P).T.astype(np.float32)),
            "tIrow": tcls.astype(np.float16).reshape(1, BL),
            "tcol": np.ascontiguousarray(
                up[:MW * P].reshape(MW, P).T.astype(np.float16)
            ),
        })

        n_t = n[tcls].astype(np.float64)
        fqd = fq.astype(np.float64)
        r2q = np.einsum("ij,ij->i", fqd, fqd)
        Eii = np.exp(SHIFT * r2q.astype(np.float32).astype(np.float64) - SHIFT)
        numer = (SHIFT * (np.einsum("ij,ij->i", fqd, M[tcls]) - r2q)) / n_t - SHIFT
        host.append({"n_t": n_t, "Eii": Eii, "numer": numer})
    return in_maps, host


_NC_CACHE = {}


def _get_nc():
    if "nc" not in _NC_CACHE:
        _NC_CACHE["nc"] = build_nc()
    return _NC_CACHE["nc"]


def run(centers1, features, targets, trace=False):
    n_cores = 8
    nc = _get_nc()
    in_maps, host = prep_inputs(centers1, features, targets, n_cores)
    res = run_bass_kernel_spmd(nc, in_maps, list(range(n_cores)), trace=trace)
    mlps = []
    for k in range(n_cores):
        out = res.results[k]["out"].astype(np.float64).reshape(-1)  # [2*BL]
        S = out[0:BL]
        Pm = out[BL:2 * BL]
        h = host[k]
        Sfix = S + (Pm - h["Eii"]) / h["n_t"]
        mlps.append(h["numer"] - np.log(Sfix))
    loss = -np.mean(np.concatenate(mlps))
    return np.float32(loss), res


def kernel(centers1, features, targets):
    loss, _ = run(centers1, features, targets)
    return np.asarray(loss, dtype=np.float32)


# revision 12
# speedup vs baseline: 1.0470x; 1.0470x over previous
"""Supervised-contrastive loss (balanced softmax variant) on 8 Trainium2 cores.

Transposed/class-sorted formulation. Rows are sorted by target class and
split 1024 per core; columns (all 8192 features + 1000 centers, merged and
class-sorted) are permuted per core so the core's "positive window" (all
columns whose class appears among its rows, <= 1280 of 9216) comes first.

Per j-tile (128 columns on partitions, 1024 rows on the free axis):
    PE : d[j, i] = A_j . f_i                    fp16 matmul, 2 x N=512
    ACT: E'[j, i] = exp(10*d + (ln a_j - 10))   a_j = 1/cls_count (the
         per-COLUMN balanced-softmax weight rides in the per-PARTITION
         activation bias -- no bias matmul at all)
    DVE: Acc[t%2] += E'          fp16 tensor_tensor add (2x_1p mode)
    DVE (window tiles only): MAcc += (tcol_j == trow_i) * E'

Final partition-reduction on the PE (ones-column matmuls) produces
S = colsum(Acc0+Acc1) and P = colsum(MAcc) as [1, 1024] rows; one 8 KB
DMA returns them. The host finishes in f64:
    S'_i  = S_i + (P_i - Eii)/n_i     n_i = bincount(targets)[t_i]
    mlp_i = 10*(f_i.M[t_i] - r2_i)/n_i - 10 - log(S'_i)
    loss  = -mean(mlp_i)
where Eii = exp(10*r2q_i - 10) removes the self column exactly (r2q is the
self dot in the same fp16 values the PE saw), and M[c] is the class sum of
fp16 features + center.
"""

import sys
from contextlib import ExitStack

import numpy as np
import ml_dtypes

sys.path.insert(0, "/opt/trn_rl_repo")

import concourse.bass as bass  # noqa: E402
import concourse.mybir as mybir  # noqa: E402
import concourse.tile as tile  # noqa: E402
from concourse import bacc  # noqa: E402
from concourse.bass_utils import run_bass_kernel_spmd  # noqa: E402

P = 128
BL = 1024          # rows per core
NT = 72            # j-tiles of 128 columns (9216 padded)
JP = NT * P
MW = 10            # masked-window j-tiles (1280 cols >= max window)
TEMP = 0.1
SHIFT = 10.0

PSUM_BUFS = 3
# fTq DMA chunk sizes in j-tiles: small first so the PE can start early
WCHUNKS = [1] * 4 + [2] * 4 + [4] * 3 + [6] * 8
assert sum(WCHUNKS) == NT
FCHUNK = 8         # featq DMA chunks

F16 = mybir.dt.float16
F32 = mybir.dt.float32
AF = mybir.ActivationFunctionType
ALU = mybir.AluOpType


def build_nc() -> bass.Bass:
    """One-core program; run SPMD on 8 cores with per-core inputs."""
    nc = bacc.Bacc(None)
    featq_d = nc.declare_dram_parameter("featq", [P, BL], F16, isOutput=False)
    fTq_d = nc.declare_dram_parameter("fTq", [P, NT * P], F16, isOutput=False)
    bias_d = nc.declare_dram_parameter("bias", [P, NT], F32, isOutput=False)
    tIrow_d = nc.declare_dram_parameter("tIrow", [1, BL], F16, isOutput=False)
    tcol_d = nc.declare_dram_parameter("tcol", [P, MW], F16, isOutput=False)
    out_d = nc.declare_dram_parameter("out", [1, 2 * BL], F32, isOutput=True)

    with tile.TileContext(nc) as tc, ExitStack() as ctx:
        const = ctx.enter_context(tc.tile_pool(name="const", bufs=1))
        epool = ctx.enter_context(tc.tile_pool(name="epool", bufs=3))
        mpool = ctx.enter_context(tc.tile_pool(name="mpool", bufs=2))
        psum = ctx.enter_context(
            tc.tile_pool(name="psum", bufs=PSUM_BUFS, space=bass.MemorySpace.PSUM)
        )
        rpsum = ctx.enter_context(
            tc.tile_pool(name="rpsum", bufs=1, space=bass.MemorySpace.PSUM)
        )

        # warm the ACT exp table while DMAs are in flight
        warm = const.tile([P, 1], F32)
        nc.vector.memset(warm[:], 0.0)
        nc.scalar.activation(warm[:], warm[:], AF.Exp, bias=warm[:], scale=1.0)

        # -- input DMAs; issue split between the Sync and GpSimd sequencers
        # (each dma_start costs ~600ns of serial issue time per sequencer).
        # Priority order: what the first loop iterations need comes first.
        featq = const.tile([P, BL], F16)
        fTqs = []

        def ftq_dma(eng, ci):
            off = sum(WCHUNKS[:ci])
            w = WCHUNKS[ci]
            ft = const.tile([P, w * P], F16, tag=f"fTq{ci}")
            eng.dma_start(ft[:], fTq_d[:, off * P:(off + w) * P])
            fTqs.append((off, w, ft))

        FS = BL // FCHUNK
        ftq_dma(nc.sync, 0)
        for c in range(FCHUNK):
            nc.sync.dma_start(featq[:, c * FS:(c + 1) * FS],
                              featq_d[:, c * FS:(c + 1) * FS])
            if c % 2 == 1 and c // 2 + 1 < 6:
                ftq_dma(nc.sync, c // 2 + 1)

        bias = const.tile([P, NT], F32)
        nc.gpsimd.dma_start(bias[:], bias_d[:])
        tcol = const.tile([P, MW], F16)
        nc.gpsimd.dma_start(tcol[:], tcol_d[:])
        tIrow_r = const.tile([1, BL], F16)
        nc.gpsimd.dma_start(tIrow_r[:], tIrow_d[:])
        tIrow = const.tile([P, BL], F16)
        nc.gpsimd.partition_broadcast(tIrow[:], tIrow_r[:])
        for ci in range(5, len(WCHUNKS)):
            ftq_dma(nc.gpsimd, ci)
        fTqs.sort(key=lambda x: x[0])

        acc0 = const.tile([P, BL], F16)
        nc.vector.memset(acc0[:], 0.0)
        acc1 = const.tile([P, BL], F16)
        nc.vector.memset(acc1[:], 0.0)
        macc = const.tile([P, BL], F16)
        nc.vector.memset(macc[:], 0.0)
        ones = const.tile([P, 1], F16)
        nc.vector.memset(ones[:], 1.0)
        accs = [acc0, acc1]

        outt = const.tile([1, 2 * BL], F32)

        def jtile(t):
            for off, w, ft in fTqs:
                if off <= t < off + w:
                    return ft[:, (t - off) * P:(t - off + 1) * P]
            raise AssertionError

        for t in range(NT):
            lhs = jtile(t)
            pt = psum.tile([P, BL], F32, tag="pt")
            for h in range(2):
                nc.tensor.matmul(
                    pt[:, h * 512:(h + 1) * 512], lhs,
                    featq[:, h * 512:(h + 1) * 512],
                    start=True, stop=True,
                )
            et = epool.tile([P, BL], F16, tag="et")
            nc.scalar.activation(
                et[:], pt[:], AF.Exp, bias=bias[:, t:t + 1], scale=SHIFT,
            )
            a = accs[t % 2]
            nc.vector.tensor_tensor(a[:], a[:], et[:], ALU.add)
            if t < MW:
                mt = mpool.tile([P, BL], F16, tag="mt")
                nc.vector.scalar_tensor_tensor(
                    out=mt[:], in0=tIrow[:], scalar=tcol[:, t:t + 1], in1=et[:],
                    op0=ALU.is_equal, op1=ALU.mult,
                )
                nc.vector.tensor_tensor(macc[:], macc[:], mt[:], ALU.add)
            if t == MW:
                # masked accumulator is final: reduce + stage its output early
                pred = rpsum.tile([1, BL], F32, tag="red")
                for h in range(2):
                    nc.tensor.matmul(pred[:, h * 512:(h + 1) * 512], ones[:],
                                     macc[:, h * 512:(h + 1) * 512],
                                     start=True, stop=True)
                nc.scalar.copy(outt[:, BL:2 * BL], pred[:])

        sred = rpsum.tile([1, BL], F32, tag="red")
        for h in range(2):
            nc.tensor.matmul(sred[:, h * 512:(h + 1) * 512], ones[:],
                             acc0[:, h * 512:(h + 1) * 512],
                             start=True, stop=False)
        for h in range(2):
            nc.tensor.matmul(sred[:, h * 512:(h + 1) * 512], ones[:],
                             acc1[:, h * 512:(h + 1) * 512],
                             start=False, stop=True)
            nc.scalar.copy(outt[:, h * 512:(h + 1) * 512],
                           sred[:, h * 512:(h + 1) * 512])
        nc.sync.dma_start(out_d[:], outt[:])

    nc.finalize()
    return nc


def prep_inputs(centers1, features, targets, n_cores):
    """Host-side sort/shard/layout prep. Returns per-core input maps and
    the per-core host epilogue data."""
    B, D = features.shape
    C = centers1.shape[0]
    J = B + C
    assert BL * n_cores == B and D == P and JP >= J

    feats16 = np.asarray(features, np.float32).astype(np.float16)
    cents16 = np.asarray(centers1, np.float32).astype(np.float16)
    targets = np.asarray(targets).astype(np.int64)

    n = np.bincount(targets, minlength=C).astype(np.int64)
    cc = n + 1

    order = np.argsort(targets, kind="stable")          # rows sorted by class
    # merged class-sorted columns: per class, feature rows then the center
    col_order = np.lexsort((
        np.concatenate([np.arange(B), np.full(C, B)]),
        np.concatenate([targets, np.arange(C)]),
    ))
    A16 = np.concatenate([feats16, cents16], axis=0)[col_order]   # [J, D]
    col_cls = np.concatenate([targets, np.arange(C)])[col_order]  # [J]
    a_col = 1.0 / cc[col_cls]

    # class sums for the numerator (f64 on the fp16 values)
    M = np.zeros((C, D))
    np.add.at(M, targets, feats16.astype(np.float64))
    M += cents16

    in_maps, host = [], []
    for k in range(n_cores):
        rids = order[k * BL:(k + 1) * BL]
        tcls = targets[rids]
        jlo = np.searchsorted(col_cls, tcls[0], "left")
        jhi = np.searchsorted(col_cls, tcls[-1], "right")
        assert jhi - jlo <= MW * P, f"core {k}: window {jhi - jlo} > {MW * P}"
        perm = np.concatenate([
            np.arange(jlo, jhi), np.arange(0, jlo), np.arange(jhi, J)
        ])

        Ap = np.zeros((JP, D), np.float16)
        Ap[:J] = A16[perm]
        up = np.full(JP, -1.0)
        up[:J] = col_cls[perm]
        biasv = np.full(JP, -200.0, np.float64)
        biasv[:J] = np.log(a_col[perm]) - SHIFT

        fq = feats16[rids]
        fTq = np.ascontiguousarray(
            Ap.reshape(NT, P, P).transpose(2, 0, 1).reshape(P, NT * P))
        featq = np.ascontiguousarray(fq.T)

        in_maps.append({
            "featq": featq,
            "fTq": fTq,
            "bias": np.ascontiguousarray(biasv.reshape(NT, P).T.astype(np.float32)),
            "tIrow": tcls.astype(np.float16).reshape(1, BL),
            "tcol": np.ascontiguousarray(
                up[:MW * P].reshape(MW, P).T.astype(np.float16)
            ),
        })

        n_t = n[tcls].astype(np.float64)
        fqd = fq.astype(np.float64)
        r2q = np.einsum("ij,ij->i", fqd, fqd)
        Eii = np.exp(SHIFT * r2q.astype(np.float32).astype(np.float64) - SHIFT)
        numer = (SHIFT * (np.einsum("ij,ij->i", fqd, M[tcls]) - r2q)) / n_t - SHIFT
        host.append({"n_t": n_t, "Eii": Eii, "numer": numer})
    return in_maps, host


_NC_CACHE = {}


def _get_nc():
    if "nc" not in _NC_CACHE:
        _NC_CACHE["nc"] = build_nc()
    return _NC_CACHE["nc"]


def run(centers1, features, targets, trace=False):
    n_cores = 8
    nc = _get_nc()
    in_maps, host = prep_inputs(centers1, features, targets, n_cores)
    res = run_bass_kernel_spmd(nc, in_maps, list(range(n_cores)), trace=trace)
    mlps = []
    for k in range(n_cores):
        out = res.results[k]["out"].astype(np.float64).reshape(-1)  # [2*BL]
        S = out[0:BL]
        Pm = out[BL:2 * BL]
        h = host[k]
        Sfix = S + (Pm - h["Eii"]) / h["n_t"]
        mlps.append(h["numer"] - np.log(Sfix))
    loss = -np.mean(np.concatenate(mlps))
    return np.float32(loss), res


def kernel(centers1, features, targets):
    loss, _ = run(centers1, features, targets)
    return np.asarray(loss, dtype=np.float32)


# revision 16
# speedup vs baseline: 1.1995x; 1.1456x over previous
"""Supervised-contrastive loss (balanced softmax variant) on 8 Trainium2 cores.

Transposed/class-sorted formulation. Rows are sorted by target class and
split 1024 per core; columns (all 8192 features + 1000 centers, merged and
class-sorted) are permuted per core so the core's "positive window" (all
columns whose class appears among its rows, <= 1280 of 9216) comes first.

Per j-tile (128 columns on partitions, 1024 rows on the free axis):
    PE : d[j, i] = A_j . f_i                    fp16 matmul, 2 x N=512
    ACT: E'[j, i] = exp(10*d + (ln a_j - 10))   a_j = 1/cls_count (the
         per-COLUMN balanced-softmax weight rides in the per-PARTITION
         activation bias -- no bias matmul at all)
    DVE: Acc[t%2] += E'          fp16 tensor_tensor add (2x_1p mode)
    DVE (window tiles only): MAcc += (tcol_j == trow_i) * E'

Final partition-reduction on the PE (ones-column matmuls) produces
S = colsum(Acc0+Acc1) and P = colsum(MAcc) as [1, 1024] rows; one 8 KB
DMA returns them. The host finishes in f64:
    S'_i  = S_i + (P_i - Eii)/n_i     n_i = bincount(targets)[t_i]
    mlp_i = 10*(f_i.M[t_i] - r2_i)/n_i - 10 - log(S'_i)
    loss  = -mean(mlp_i)
where Eii = exp(10*r2q_i - 10) removes the self column exactly (r2q is the
self dot in the same fp16 values the PE saw), and M[c] is the class sum of
fp16 features + center.
"""

import sys
from contextlib import ExitStack

import numpy as np
import ml_dtypes

sys.path.insert(0, "/opt/trn_rl_repo")

import concourse.bass as bass  # noqa: E402
import concourse.mybir as mybir  # noqa: E402
import concourse.tile as tile  # noqa: E402
from concourse import bacc  # noqa: E402
from concourse.bass_utils import run_bass_kernel_spmd  # noqa: E402

P = 128
BL = 1024          # rows per core
NT = 72            # j-tiles of 128 columns (9216 padded)
JP = NT * P
MW = 10            # masked-window j-tiles (1280 cols >= max window)
TEMP = 0.1
SHIFT = 10.0

PSUM_BUFS = 3
# fTq DMA chunk sizes in j-tiles: small first so the PE can start early
WCHUNKS = [1] * 4 + [2] * 4 + [4] * 3 + [6] * 8
assert sum(WCHUNKS) == NT
FCHUNK = 4         # featq DMA chunks
# processing slot of each masked (positive-window) j-tile: spread them out
# so the extra DVE work (stt + add) stays under the ACT exp pace
MSLOTS = [3 + 7 * m for m in range(MW)]

F16 = mybir.dt.float16
F32 = mybir.dt.float32
AF = mybir.ActivationFunctionType
ALU = mybir.AluOpType


def build_nc() -> bass.Bass:
    """One-core program; run SPMD on 8 cores with per-core inputs."""
    nc = bacc.Bacc(None)
    featq_d = nc.declare_dram_parameter("featq", [P, BL], F16, isOutput=False)
    fTq_d = nc.declare_dram_parameter("fTq", [P, NT * P], F16, isOutput=False)
    bias_d = nc.declare_dram_parameter("bias", [P, NT], F32, isOutput=False)
    tIrow_d = nc.declare_dram_parameter("tIrow", [1, BL], F16, isOutput=False)
    tcol_d = nc.declare_dram_parameter("tcol", [P, MW], F16, isOutput=False)
    out_d = nc.declare_dram_parameter("out", [1, 2 * BL], F32, isOutput=True)

    with tile.TileContext(nc) as tc, ExitStack() as ctx:
        const = ctx.enter_context(tc.tile_pool(name="const", bufs=1))
        epool = ctx.enter_context(tc.tile_pool(name="epool", bufs=3))
        mpool = ctx.enter_context(tc.tile_pool(name="mpool", bufs=2))
        psum = ctx.enter_context(
            tc.tile_pool(name="psum", bufs=PSUM_BUFS, space=bass.MemorySpace.PSUM)
        )
        rpsum = ctx.enter_context(
            tc.tile_pool(name="rpsum", bufs=1, space=bass.MemorySpace.PSUM)
        )

        # warm the ACT exp table while DMAs are in flight
        warm = const.tile([P, 1], F32)
        nc.vector.memset(warm[:], 0.0)
        nc.scalar.activation(warm[:], warm[:], AF.Exp, bias=warm[:], scale=1.0)

        # -- input DMAs; issue split between the Sync and GpSimd sequencers
        # (each dma_start costs ~600ns of serial issue time per sequencer).
        # Priority order: what the first loop iterations need comes first.
        featq = const.tile([P, BL], F16)
        fTqs = []

        def ftq_dma(eng, ci):
            off = sum(WCHUNKS[:ci])
            w = WCHUNKS[ci]
            ft = const.tile([P, w * P], F16, tag=f"fTq{ci}")
            eng.dma_start(ft[:], fTq_d[:, off * P:(off + w) * P])
            fTqs.append((off, w, ft))

        FS = BL // FCHUNK
        ftq_dma(nc.sync, 0)
        for c in range(FCHUNK):
            nc.sync.dma_start(featq[:, c * FS:(c + 1) * FS],
                              featq_d[:, c * FS:(c + 1) * FS])
            if c + 1 < 5:
                ftq_dma(nc.sync, c + 1)
        for ci in range(5, 9):
            ftq_dma(nc.sync, ci)

        tIrow_r = const.tile([1, BL], F16)
        nc.gpsimd.dma_start(tIrow_r[:], tIrow_d[:])
        bias = const.tile([P, NT], F32)
        nc.gpsimd.dma_start(bias[:], bias_d[:])
        tcol = const.tile([P, MW], F16)
        nc.gpsimd.dma_start(tcol[:], tcol_d[:])
        tIrow = const.tile([P, BL], F16)
        nc.gpsimd.partition_broadcast(tIrow[:], tIrow_r[:])
        for ci in range(9, len(WCHUNKS)):
            ftq_dma(nc.gpsimd, ci)
        fTqs.sort(key=lambda x: x[0])

        acc0 = const.tile([P, BL], F16)
        nc.vector.memset(acc0[:], 0.0)
        acc1 = const.tile([P, BL], F16)
        nc.vector.memset(acc1[:], 0.0)
        macc = const.tile([P, BL], F16)
        nc.vector.memset(macc[:], 0.0)
        ones = const.tile([P, 1], F16)
        nc.vector.memset(ones[:], 1.0)
        accs = [acc0, acc1]

        outt = const.tile([1, 2 * BL], F32)

        def jtile(t):
            for off, w, ft in fTqs:
                if off <= t < off + w:
                    return ft[:, (t - off) * P:(t - off + 1) * P]
            raise AssertionError

        for t in range(NT):
            lhs = jtile(t)
            pt = psum.tile([P, BL], F32, tag="pt")
            for h in range(2):
                nc.tensor.matmul(
                    pt[:, h * 512:(h + 1) * 512], lhs,
                    featq[:, h * 512:(h + 1) * 512],
                    start=True, stop=True,
                )
            et = epool.tile([P, BL], F16, tag="et")
            nc.scalar.activation(
                et[:], pt[:], AF.Exp, bias=bias[:, t:t + 1], scale=SHIFT,
            )
            a = accs[t % 2]
            nc.vector.tensor_tensor(a[:], a[:], et[:], ALU.add)
            if t in MSLOTS:
                m = MSLOTS.index(t)
                mt = mpool.tile([P, BL], F16, tag="mt")
                nc.vector.scalar_tensor_tensor(
                    out=mt[:], in0=tIrow[:], scalar=tcol[:, m:m + 1], in1=et[:],
                    op0=ALU.is_equal, op1=ALU.mult,
                )
                nc.vector.tensor_tensor(macc[:], macc[:], mt[:], ALU.add)
            if t == MSLOTS[-1] + 1:
                # masked accumulator is final: reduce + stage its output early
                pred = rpsum.tile([1, BL], F32, tag="red")
                for h in range(2):
                    nc.tensor.matmul(pred[:, h * 512:(h + 1) * 512], ones[:],
                                     macc[:, h * 512:(h + 1) * 512],
                                     start=True, stop=True)
                nc.scalar.copy(outt[:, BL:2 * BL], pred[:])

        sred = rpsum.tile([1, BL], F32, tag="red")
        for h in range(2):
            nc.tensor.matmul(sred[:, h * 512:(h + 1) * 512], ones[:],
                             acc0[:, h * 512:(h + 1) * 512],
                             start=True, stop=False)
        for h in range(2):
            nc.tensor.matmul(sred[:, h * 512:(h + 1) * 512], ones[:],
                             acc1[:, h * 512:(h + 1) * 512],
                             start=False, stop=True)
            nc.scalar.copy(outt[:, h * 512:(h + 1) * 512],
                           sred[:, h * 512:(h + 1) * 512])
        nc.sync.dma_start(out_d[:], outt[:])

    nc.finalize()
    return nc


def prep_inputs(centers1, features, targets, n_cores):
    """Host-side sort/shard/layout prep. Returns per-core input maps and
    the per-core host epilogue data."""
    B, D = features.shape
    C = centers1.shape[0]
    J = B + C
    assert BL * n_cores == B and D == P and JP >= J

    feats16 = np.asarray(features, np.float32).astype(np.float16)
    cents16 = np.asarray(centers1, np.float32).astype(np.float16)
    targets = np.asarray(targets).astype(np.int64)

    n = np.bincount(targets, minlength=C).astype(np.int64)
    cc = n + 1

    order = np.argsort(targets, kind="stable")          # rows sorted by class
    # merged class-sorted columns: per class, feature rows then the center
    col_order = np.lexsort((
        np.concatenate([np.arange(B), np.full(C, B)]),
        np.concatenate([targets, np.arange(C)]),
    ))
    A16 = np.concatenate([feats16, cents16], axis=0)[col_order]   # [J, D]
    col_cls = np.concatenate([targets, np.arange(C)])[col_order]  # [J]
    a_col = 1.0 / cc[col_cls]

    # class sums for the numerator (f64 on the fp16 values)
    M = np.zeros((C, D))
    np.add.at(M, targets, feats16.astype(np.float64))
    M += cents16

    in_maps, host = [], []
    for k in range(n_cores):
        rids = order[k * BL:(k + 1) * BL]
        tcls = targets[rids]
        jlo = np.searchsorted(col_cls, tcls[0], "left")
        jhi = np.searchsorted(col_cls, tcls[-1], "right")
        assert jhi - jlo <= MW * P, f"core {k}: window {jhi - jlo} > {MW * P}"
        perm = np.concatenate([
            np.arange(jlo, jhi), np.arange(0, jlo), np.arange(jhi, J)
        ])

        Ap = np.zeros((JP, D), np.float16)
        Ap[:J] = A16[perm]
        up = np.full(JP, -1.0)
        up[:J] = col_cls[perm]
        biasv = np.full(JP, -200.0, np.float64)
        biasv[:J] = np.log(a_col[perm]) - SHIFT

        # scatter the column groups to processing slots: window tile m goes
        # to slot MSLOTS[m], the rest fill the remaining slots in order
        slot_of = np.empty(NT, np.int64)        # slot -> group index
        rest = [s for s in range(NT) if s not in MSLOTS]
        for m, s in enumerate(MSLOTS):
            slot_of[s] = m
        for g, s in enumerate(rest):
            slot_of[s] = MW + g
        cperm = (slot_of[:, None] * P + np.arange(P)[None, :]).reshape(-1)
        Ap = Ap[cperm]
        biasv = biasv[cperm]

        fq = feats16[rids]
        fTq = np.ascontiguousarray(
            Ap.reshape(NT, P, P).transpose(2, 0, 1).reshape(P, NT * P))
        featq = np.ascontiguousarray(fq.T)

        in_maps.append({
            "featq": featq,
            "fTq": fTq,
            "bias": np.ascontiguousarray(biasv.reshape(NT, P).T.astype(np.float32)),
            "tIrow": tcls.astype(np.float16).reshape(1, BL),
            "tcol": np.ascontiguousarray(
                up[:MW * P].reshape(MW, P).T.astype(np.float16)
            ),
        })

        n_t = n[tcls].astype(np.float64)
        fqd = fq.astype(np.float64)
        r2q = np.einsum("ij,ij->i", fqd, fqd)
        Eii = np.exp(SHIFT * r2q.astype(np.float32).astype(np.float64) - SHIFT)
        numer = (SHIFT * (np.einsum("ij,ij->i", fqd, M[tcls]) - r2q)) / n_t - SHIFT
        host.append({"n_t": n_t, "Eii": Eii, "numer": numer})
    return in_maps, host


_NC_CACHE = {}


def _get_nc():
    if "nc" not in _NC_CACHE:
        _NC_CACHE["nc"] = build_nc()
    return _NC_CACHE["nc"]


def run(centers1, features, targets, trace=False):
    n_cores = 8
    nc = _get_nc()
    in_maps, host = prep_inputs(centers1, features, targets, n_cores)
    res = run_bass_kernel_spmd(nc, in_maps, list(range(n_cores)), trace=trace)
    mlps = []
    for k in range(n_cores):
        out = res.results[k]["out"].astype(np.float64).reshape(-1)  # [2*BL]
        S = out[0:BL]
        Pm = out[BL:2 * BL]
        h = host[k]
        Sfix = S + (Pm - h["Eii"]) / h["n_t"]
        mlps.append(h["numer"] - np.log(Sfix))
    loss = -np.mean(np.concatenate(mlps))
    return np.float32(loss), res


def kernel(centers1, features, targets):
    loss, _ = run(centers1, features, targets)
    return np.asarray(loss, dtype=np.float32)


# revision 17
# speedup vs baseline: 1.2066x; 1.0060x over previous
"""Supervised-contrastive loss (balanced softmax variant) on 8 Trainium2 cores.

Transposed/class-sorted formulation. Rows are sorted by target class and
split 1024 per core; columns (all 8192 features + 1000 centers, merged and
class-sorted) are permuted per core so the core's "positive window" (all
columns whose class appears among its rows, <= 1280 of 9216) comes first,
then scattered so one masked j-tile lands every 7th processing slot (the
masked tiles cost extra DVE work; spreading them keeps DVE under the ACT
exp pace).

Per j-tile (128 columns on partitions, 1024 rows on the free axis):
    PE : d[j, i] = A_j . f_i                    fp8(e4m3) matmul, 2 x N=512
    ACT: E'[j, i] = exp(10*d + (ln a_j - 10))   a_j = 1/cls_count (the
         per-COLUMN balanced-softmax weight rides in the per-PARTITION
         activation bias -- no bias matmul at all)
    DVE: Acc[t%2] += E'          fp16 tensor_tensor add (2x_1p mode)
    DVE (masked tiles only): MAcc += (tcol_j == trow_i) * E'

Final partition-reduction on the PE (ones-column matmuls) produces
S = colsum(Acc0+Acc1) and P = colsum(MAcc) as [1, 1024] rows; one 8 KB
DMA returns them. The host finishes in f64:
    S'_i  = S_i + (P_i - Eii)/n_i     n_i = bincount(targets)[t_i]
    mlp_i = 10*(f_i.M[t_i] - r2_i)/n_i - 10 - log(S'_i)
    loss  = -mean(mlp_i)
where Eii = exp(10*r2q_i - 10) removes the self column exactly (r2q is the
self dot in the same fp8 values the PE saw), and M[c] is the class sum of
fp16 features + center.
"""

import sys
from contextlib import ExitStack

import numpy as np
import ml_dtypes

sys.path.insert(0, "/opt/trn_rl_repo")

import concourse.bass as bass  # noqa: E402
import concourse.mybir as mybir  # noqa: E402
import concourse.tile as tile  # noqa: E402
from concourse import bacc  # noqa: E402
from concourse.bass_utils import run_bass_kernel_spmd  # noqa: E402

P = 128
BL = 1024          # rows per core
NT = 72            # j-tiles of 128 columns (9216 padded)
JP = NT * P
MW = 10            # masked-window j-tiles (1280 cols >= max window)
TEMP = 0.1
SHIFT = 10.0

PSUM_BUFS = 3
# fTq DMA chunk sizes in j-tiles: small first so the PE can start early
WCHUNKS = [1] * 4 + [2] * 4 + [4] * 3 + [6] * 8
assert sum(WCHUNKS) == NT
FCHUNK = 4         # featq DMA chunks
# processing slot of each masked (positive-window) j-tile
MSLOTS = [3 + 7 * m for m in range(MW)]

F8NP = ml_dtypes.float8_e4m3
F8 = mybir.dt.float8e4
F16 = mybir.dt.float16
F32 = mybir.dt.float32
AF = mybir.ActivationFunctionType
ALU = mybir.AluOpType


def build_nc() -> bass.Bass:
    """One-core program; run SPMD on 8 cores with per-core inputs."""
    nc = bacc.Bacc(None)
    # per-chunk weight/feature params: each is a contiguous row-major block
    # in DRAM so its DMA is one linear burst per partition run
    featq_ds = [
        nc.declare_dram_parameter(f"featq{c}", [P, BL // FCHUNK], F8,
                                  isOutput=False)
        for c in range(FCHUNK)
    ]
    fTq_ds = [
        nc.declare_dram_parameter(f"fTq{ci}", [P, w * P], F8, isOutput=False)
        for ci, w in enumerate(WCHUNKS)
    ]
    bias_d = nc.declare_dram_parameter("bias", [P, NT], F32, isOutput=False)
    tIrow_d = nc.declare_dram_parameter("tIrow", [1, BL], F16, isOutput=False)
    tcol_d = nc.declare_dram_parameter("tcol", [P, MW], F16, isOutput=False)
    out_d = nc.declare_dram_parameter("out", [1, 2 * BL], F32, isOutput=True)

    with tile.TileContext(nc) as tc, ExitStack() as ctx:
        const = ctx.enter_context(tc.tile_pool(name="const", bufs=1))
        epool = ctx.enter_context(tc.tile_pool(name="epool", bufs=3))
        mpool = ctx.enter_context(tc.tile_pool(name="mpool", bufs=2))
        psum = ctx.enter_context(
            tc.tile_pool(name="psum", bufs=PSUM_BUFS, space=bass.MemorySpace.PSUM)
        )
        rpsum = ctx.enter_context(
            tc.tile_pool(name="rpsum", bufs=1, space=bass.MemorySpace.PSUM)
        )

        # warm the ACT exp table while DMAs are in flight
        warm = const.tile([P, 1], F32)
        nc.vector.memset(warm[:], 0.0)
        nc.scalar.activation(warm[:], warm[:], AF.Exp, bias=warm[:], scale=1.0)

        # -- input DMAs; issue split between the Sync and GpSimd sequencers
        # (each dma_start costs ~600ns of serial issue time per sequencer).
        # Priority order: what the first loop iterations need comes first.
        featq = const.tile([P, BL], F8)
        fTqs = []

        def ftq_dma(eng, ci):
            off = sum(WCHUNKS[:ci])
            w = WCHUNKS[ci]
            ft = const.tile([P, w * P], F8, tag=f"fTq{ci}")
            eng.dma_start(ft[:], fTq_ds[ci][:])
            fTqs.append((off, w, ft))

        FS = BL // FCHUNK
        ftq_dma(nc.sync, 0)
        for c in range(FCHUNK):
            nc.sync.dma_start(featq[:, c * FS:(c + 1) * FS], featq_ds[c][:])
            if c + 1 < 5:
                ftq_dma(nc.sync, c + 1)
        for ci in range(5, 9):
            ftq_dma(nc.sync, ci)

        bias = const.tile([P, NT], F32)
        nc.gpsimd.dma_start(bias[:], bias_d[:])
        tIrow_r = const.tile([1, BL], F16)
        nc.gpsimd.dma_start(tIrow_r[:], tIrow_d[:])
        tcol = const.tile([P, MW], F16)
        nc.gpsimd.dma_start(tcol[:], tcol_d[:])
        tIrow = const.tile([P, BL], F16)
        nc.gpsimd.partition_broadcast(tIrow[:], tIrow_r[:])
        for ci in range(9, len(WCHUNKS)):
            ftq_dma(nc.gpsimd, ci)
        fTqs.sort(key=lambda x: x[0])

        acc0 = const.tile([P, BL], F16)
        nc.vector.memset(acc0[:], 0.0)
        acc1 = const.tile([P, BL], F16)
        nc.vector.memset(acc1[:], 0.0)
        macc = const.tile([P, BL], F16)
        nc.vector.memset(macc[:], 0.0)
        ones = const.tile([P, 1], F16)
        nc.vector.memset(ones[:], 1.0)
        accs = [acc0, acc1]

        outt = const.tile([1, 2 * BL], F32)

        def jtile(t):
            for off, w, ft in fTqs:
                if off <= t < off + w:
                    return ft[:, (t - off) * P:(t - off + 1) * P]
            raise AssertionError

        for t in range(NT):
            lhs = jtile(t)
            pt = psum.tile([P, BL], F32, tag="pt")
            for h in range(2):
                nc.tensor.matmul(
                    pt[:, h * 512:(h + 1) * 512], lhs,
                    featq[:, h * 512:(h + 1) * 512],
                    start=True, stop=True,
                )
            et = epool.tile([P, BL], F16, tag="et")
            nc.scalar.activation(
                et[:], pt[:], AF.Exp, bias=bias[:, t:t + 1], scale=SHIFT,
            )
            a = accs[t % 2]
            nc.vector.tensor_tensor(a[:], a[:], et[:], ALU.add)
            if t in MSLOTS:
                m = MSLOTS.index(t)
                mt = mpool.tile([P, BL], F16, tag="mt")
                nc.vector.scalar_tensor_tensor(
                    out=mt[:], in0=tIrow[:], scalar=tcol[:, m:m + 1], in1=et[:],
                    op0=ALU.is_equal, op1=ALU.mult,
                )
                nc.vector.tensor_tensor(macc[:], macc[:], mt[:], ALU.add)
            if t == MSLOTS[-1] + 1:
                # masked accumulator is final: reduce + stage its output early
                pred = rpsum.tile([1, BL], F32, tag="red")
                for h in range(2):
                    nc.tensor.matmul(pred[:, h * 512:(h + 1) * 512], ones[:],
                                     macc[:, h * 512:(h + 1) * 512],
                                     start=True, stop=True)
                nc.scalar.copy(outt[:, BL:2 * BL], pred[:])

        sred = rpsum.tile([1, BL], F32, tag="red")
        for h in range(2):
            nc.tensor.matmul(sred[:, h * 512:(h + 1) * 512], ones[:],
                             acc0[:, h * 512:(h + 1) * 512],
                             start=True, stop=False)
        for h in range(2):
            nc.tensor.matmul(sred[:, h * 512:(h + 1) * 512], ones[:],
                             acc1[:, h * 512:(h + 1) * 512],
                             start=False, stop=True)
            nc.scalar.copy(outt[:, h * 512:(h + 1) * 512],
                           sred[:, h * 512:(h + 1) * 512])
        nc.sync.dma_start(out_d[:], outt[:])

    nc.finalize()
    return nc


def prep_inputs(centers1, features, targets, n_cores):
    """Host-side sort/shard/layout prep. Returns per-core input maps and
    the per-core host epilogue data."""
    B, D = features.shape
    C = centers1.shape[0]
    J = B + C
    assert BL * n_cores == B and D == P and JP >= J

    feats16 = np.asarray(features, np.float32).astype(np.float16)
    cents16 = np.asarray(centers1, np.float32).astype(np.float16)
    targets = np.asarray(targets).astype(np.int64)

    n = np.bincount(targets, minlength=C).astype(np.int64)
    cc = n + 1

    order = np.argsort(targets, kind="stable")          # rows sorted by class
    # merged class-sorted columns: per class, feature rows then the center
    col_order = np.lexsort((
        np.concatenate([np.arange(B), np.full(C, B)]),
        np.concatenate([targets, np.arange(C)]),
    ))
    A16 = np.concatenate([feats16, cents16], axis=0)[col_order]   # [J, D]
    col_cls = np.concatenate([targets, np.arange(C)])[col_order]  # [J]
    a_col = 1.0 / cc[col_cls]

    # class sums for the numerator (f64 on the fp16 values)
    M = np.zeros((C, D))
    np.add.at(M, targets, feats16.astype(np.float64))
    M += cents16

    in_maps, host = [], []
    for k in range(n_cores):
        rids = order[k * BL:(k + 1) * BL]
        tcls = targets[rids]
        jlo = np.searchsorted(col_cls, tcls[0], "left")
        jhi = np.searchsorted(col_cls, tcls[-1], "right")
        assert jhi - jlo <= MW * P, f"core {k}: window {jhi - jlo} > {MW * P}"
        perm = np.concatenate([
            np.arange(jlo, jhi), np.arange(0, jlo), np.arange(jhi, J)
        ])

        Ap = np.zeros((JP, D), np.float16)
        Ap[:J] = A16[perm]
        up = np.full(JP, -1.0)
        up[:J] = col_cls[perm]
        biasv = np.full(JP, -200.0, np.float64)
        biasv[:J] = np.log(a_col[perm]) - SHIFT

        # scatter the column groups to processing slots: window tile m goes
        # to slot MSLOTS[m], the rest fill the remaining slots in order
        slot_of = np.empty(NT, np.int64)        # slot -> group index
        rest = [s for s in range(NT) if s not in MSLOTS]
        for m, s in enumerate(MSLOTS):
            slot_of[s] = m
        for g, s in enumerate(rest):
            slot_of[s] = MW + g
        cperm = (slot_of[:, None] * P + np.arange(P)[None, :]).reshape(-1)
        Ap = Ap[cperm]
        biasv = biasv[cperm]

        fq8 = feats16[rids].astype(F8NP)         # [BL, 128]
        A8 = Ap.astype(F8NP)                     # [JP, 128]

        im = {
            "bias": np.ascontiguousarray(biasv.reshape(NT, P).T.astype(np.float32)),
            "tIrow": tcls.astype(np.float16).reshape(1, BL),
            "tcol": np.ascontiguousarray(
                up[:MW * P].reshape(MW, P).T.astype(np.float16)
            ),
        }
        featq = np.ascontiguousarray(fq8.T)       # [128, BL]
        FS = BL // FCHUNK
        for c in range(FCHUNK):
            im[f"featq{c}"] = np.ascontiguousarray(featq[:, c * FS:(c + 1) * FS])
        fTq = A8.reshape(NT, P, P).transpose(2, 0, 1).reshape(P, NT * P)
        off = 0
        for ci, w in enumerate(WCHUNKS):
            im[f"fTq{ci}"] = np.ascontiguousarray(fTq[:, off * P:(off + w) * P])
            off += w
        in_maps.append(im)

        n_t = n[tcls].astype(np.float64)
        fqd = fq8.astype(np.float64)
        r2q = np.einsum("ij,ij->i", fqd, fqd)
        Eii = np.exp(SHIFT * r2q.astype(np.float32).astype(np.float64) - SHIFT)
        f16d = feats16[rids].astype(np.float64)
        r2n = np.einsum("ij,ij->i", f16d, f16d)
        numer = (SHIFT * (np.einsum("ij,ij->i", f16d, M[tcls]) - r2n)) / n_t - SHIFT
        host.append({"n_t": n_t, "Eii": Eii, "numer": numer})
    return in_maps, host


_NC_CACHE = {}


def _get_nc():
    if "nc" not in _NC_CACHE:
        _NC_CACHE["nc"] = build_nc()
    return _NC_CACHE["nc"]


def run(centers1, features, targets, trace=False):
    n_cores = 8
    nc = _get_nc()
    in_maps, host = prep_inputs(centers1, features, targets, n_cores)
    res = run_bass_kernel_spmd(nc, in_maps, list(range(n_cores)), trace=trace)
    mlps = []
    for k in range(n_cores):
        out = res.results[k]["out"].astype(np.float64).reshape(-1)  # [2*BL]
        S = out[0:BL]
        Pm = out[BL:2 * BL]
        h = host[k]
        Sfix = S + (Pm - h["Eii"]) / h["n_t"]
        mlps.append(h["numer"] - np.log(Sfix))
    loss = -np.mean(np.concatenate(mlps))
    return np.float32(loss), res


def kernel(centers1, features, targets):
    loss, _ = run(centers1, features, targets)
    return np.asarray(loss, dtype=np.float32)


# revision 21
# speedup vs baseline: 1.2420x; 1.0293x over previous
"""Supervised-contrastive loss (balanced softmax variant) on 8 Trainium2 cores.

Transposed/class-sorted formulation. Rows are sorted by target class and
split 1024 per core; columns (all 8192 features + 1000 centers, merged and
class-sorted) are permuted per core so the core's "positive window" (all
columns whose class appears among its rows, <= 1280 of 9216) comes first,
then scattered so one masked j-tile lands every 7th processing slot (the
masked tiles cost extra DVE work; spreading them keeps DVE under the ACT
exp pace).

Per j-tile (128 columns on partitions, 1024 rows on the free axis):
    PE : d[j, i] = A_j . f_i                    fp8(e4m3) matmul, 2 x N=512
    ACT: E'[j, i] = exp(10*d + (ln a_j - 10))   a_j = 1/cls_count (the
         per-COLUMN balanced-softmax weight rides in the per-PARTITION
         activation bias -- no bias matmul at all)
    DVE: Acc[t%2] += E'          fp16 tensor_tensor add (2x_1p mode)
    DVE (masked tiles only): MAcc += (tcol_j == trow_i) * E'

Final partition-reduction on the PE (ones-column matmuls) produces
S = colsum(Acc0+Acc1) and P = colsum(MAcc) as [1, 1024] rows; one 8 KB
DMA returns them. The host finishes in f64:
    S'_i  = S_i + (P_i - Eii)/n_i     n_i = bincount(targets)[t_i]
    mlp_i = 10*(f_i.M[t_i] - r2_i)/n_i - 10 - log(S'_i)
    loss  = -mean(mlp_i)
where Eii = exp(10*r2q_i - 10) removes the self column exactly (r2q is the
self dot in the same fp8 values the PE saw), and M[c] is the class sum of
fp16 features + center.
"""

import sys
from contextlib import ExitStack

import numpy as np
import ml_dtypes

sys.path.insert(0, "/opt/trn_rl_repo")

import concourse.bass as bass  # noqa: E402
import concourse.mybir as mybir  # noqa: E402
import concourse.tile as tile  # noqa: E402
from concourse import bacc  # noqa: E402
from concourse.bass_utils import run_bass_kernel_spmd  # noqa: E402

P = 128
BL = 1024          # rows per core
NT = 72            # j-tiles of 128 columns (9216 padded)
JP = NT * P
MW = 10            # masked-window j-tiles (1280 cols >= max window)
TEMP = 0.1
SHIFT = 10.0

PSUM_BUFS = 3
# fTq DMA chunk sizes in j-tiles: small first so the PE can start early
WCHUNKS = [1] * 4 + [2] * 4 + [4] * 3 + [6] * 8
assert sum(WCHUNKS) == NT
FCHUNK = 4         # featq DMA chunks
# processing slot of each masked (positive-window) j-tile
MSLOTS = [3 + 7 * m for m in range(MW)]

F8NP = ml_dtypes.float8_e4m3
F8 = mybir.dt.float8e4
F16 = mybir.dt.float16
F32 = mybir.dt.float32
AF = mybir.ActivationFunctionType
ALU = mybir.AluOpType


def build_nc() -> bass.Bass:
    """One-core program; run SPMD on 8 cores with per-core inputs."""
    nc = bacc.Bacc(None)
    # per-chunk weight/feature params: each is a contiguous row-major block
    # in DRAM so its DMA is one linear burst per partition run
    featq_ds = [
        nc.declare_dram_parameter(f"featq{c}", [P, BL // FCHUNK], F8,
                                  isOutput=False)
        for c in range(FCHUNK)
    ]
    fTq_ds = [
        nc.declare_dram_parameter(f"fTq{ci}", [P, w * P], F8, isOutput=False)
        for ci, w in enumerate(WCHUNKS)
    ]
    bias_d = nc.declare_dram_parameter("bias", [P, NT], F32, isOutput=False)
    tIrow_d = nc.declare_dram_parameter("tIrow", [1, BL], F16, isOutput=False)
    tcol_d = nc.declare_dram_parameter("tcol", [P, MW], F16, isOutput=False)
    out_d = nc.declare_dram_parameter("out", [1, 2 * BL], F32, isOutput=True)

    with tile.TileContext(nc) as tc, ExitStack() as ctx:
        const = ctx.enter_context(tc.tile_pool(name="const", bufs=1))
        epool = ctx.enter_context(tc.tile_pool(name="epool", bufs=4))
        mpool = ctx.enter_context(tc.tile_pool(name="mpool", bufs=2))
        psum = ctx.enter_context(
            tc.tile_pool(name="psum", bufs=PSUM_BUFS, space=bass.MemorySpace.PSUM)
        )
        rpsum = ctx.enter_context(
            tc.tile_pool(name="rpsum", bufs=1, space=bass.MemorySpace.PSUM)
        )

        # warm the ACT exp table while DMAs are in flight
        warm = const.tile([P, 1], F32)
        nc.vector.memset(warm[:], 0.0)
        nc.scalar.activation(warm[:], warm[:], AF.Exp, bias=warm[:], scale=1.0)

        # -- input DMAs; issue split between the Sync and GpSimd sequencers
        # (each dma_start costs ~600ns of serial issue time per sequencer).
        # Priority order: what the first loop iterations need comes first.
        featq = const.tile([P, BL], F8)
        fTqs = []

        def ftq_dma(eng, ci):
            off = sum(WCHUNKS[:ci])
            w = WCHUNKS[ci]
            ft = const.tile([P, w * P], F8, tag=f"fTq{ci}")
            eng.dma_start(ft[:], fTq_ds[ci][:])
            fTqs.append((off, w, ft))

        FS = BL // FCHUNK
        for c in range(FCHUNK):
            nc.sync.dma_start(featq[:, c * FS:(c + 1) * FS], featq_ds[c][:])
            ftq_dma(nc.sync, c)
        for ci in range(4, 9):
            ftq_dma(nc.sync, ci)

        bias = const.tile([P, NT], F32)
        nc.gpsimd.dma_start(bias[:, 0:8], bias_d[:, 0:8])
        tIrow_r = const.tile([1, BL], F16)
        nc.gpsimd.dma_start(tIrow_r[:], tIrow_d[:])
        tcol = const.tile([P, MW], F16)
        nc.gpsimd.dma_start(tcol[:], tcol_d[:])
        nc.gpsimd.dma_start(bias[:, 8:NT], bias_d[:, 8:NT])
        tIrow = const.tile([P, BL], F16)
        nc.gpsimd.partition_broadcast(tIrow[:], tIrow_r[:])
        for ci in range(9, len(WCHUNKS)):
            ftq_dma(nc.gpsimd, ci)
        fTqs.sort(key=lambda x: x[0])

        acc0 = const.tile([P, BL], F16)
        nc.vector.memset(acc0[:], 0.0)
        acc1 = const.tile([P, BL], F16)
        nc.vector.memset(acc1[:], 0.0)
        macc = const.tile([P, BL], F16)
        nc.vector.memset(macc[:], 0.0)
        ones = const.tile([P, 1], F16)
        nc.vector.memset(ones[:], 1.0)
        accs = [acc0, acc1]

        outt = const.tile([1, 2 * BL], F32)

        def jtile(t):
            for off, w, ft in fTqs:
                if off <= t < off + w:
                    return ft[:, (t - off) * P:(t - off + 1) * P]
            raise AssertionError

        for t in range(NT):
            lhs = jtile(t)
            pt = psum.tile([P, BL], F32, tag="pt")
            for h in range(2):
                nc.tensor.matmul(
                    pt[:, h * 512:(h + 1) * 512], lhs,
                    featq[:, h * 512:(h + 1) * 512],
                    start=True, stop=True,
                )
            et = epool.tile([P, BL], F16, tag="et")
            nc.scalar.activation(
                et[:], pt[:], AF.Exp, bias=bias[:, t:t + 1], scale=SHIFT,
            )
            a = accs[t % 2]
            nc.vector.tensor_tensor(a[:], a[:], et[:], ALU.add)
            if t in MSLOTS:
                m = MSLOTS.index(t)
                mt = mpool.tile([P, BL], F16, tag="mt")
                nc.vector.scalar_tensor_tensor(
                    out=mt[:], in0=tIrow[:], scalar=tcol[:, m:m + 1], in1=et[:],
                    op0=ALU.is_equal, op1=ALU.mult,
                )
                nc.vector.tensor_tensor(macc[:], macc[:], mt[:], ALU.add)
            if t == MSLOTS[-1] + 1:
                # masked accumulator is final: reduce + stage its output early
                pred = rpsum.tile([1, BL], F32, tag="red")
                for h in range(2):
                    nc.tensor.matmul(pred[:, h * 512:(h + 1) * 512], ones[:],
                                     macc[:, h * 512:(h + 1) * 512],
                                     start=True, stop=True)
                nc.scalar.copy(outt[:, BL:2 * BL], pred[:])
                nc.sync.dma_start(out_d[:, BL:2 * BL], outt[:, BL:2 * BL])

        sred = rpsum.tile([1, BL], F32, tag="red")
        for h in range(2):
            nc.tensor.matmul(sred[:, h * 512:(h + 1) * 512], ones[:],
                             acc0[:, h * 512:(h + 1) * 512],
                             start=True, stop=False)
        for h in range(2):
            nc.tensor.matmul(sred[:, h * 512:(h + 1) * 512], ones[:],
                             acc1[:, h * 512:(h + 1) * 512],
                             start=False, stop=True)
        nc.scalar.copy(outt[:, 0:512], sred[:, 0:512])
        nc.vector.tensor_scalar_add(outt[:, 512:BL], sred[:, 512:BL], 0.0)
        nc.sync.dma_start(out_d[:, 0:BL], outt[:, 0:BL])

    nc.finalize()
    return nc


def prep_inputs(centers1, features, targets, n_cores):
    """Host-side sort/shard/layout prep. Returns per-core input maps and
    the per-core host epilogue data."""
    B, D = features.shape
    C = centers1.shape[0]
    J = B + C
    assert BL * n_cores == B and D == P and JP >= J

    feats16 = np.asarray(features, np.float32).astype(np.float16)
    cents16 = np.asarray(centers1, np.float32).astype(np.float16)
    targets = np.asarray(targets).astype(np.int64)

    n = np.bincount(targets, minlength=C).astype(np.int64)
    cc = n + 1

    order = np.argsort(targets, kind="stable")          # rows sorted by class
    # merged class-sorted columns: per class, feature rows then the center
    col_order = np.lexsort((
        np.concatenate([np.arange(B), np.full(C, B)]),
        np.concatenate([targets, np.arange(C)]),
    ))
    A16 = np.concatenate([feats16, cents16], axis=0)[col_order]   # [J, D]
    col_cls = np.concatenate([targets, np.arange(C)])[col_order]  # [J]
    a_col = 1.0 / cc[col_cls]

    # class sums for the numerator (f64 on the fp16 values)
    M = np.zeros((C, D))
    np.add.at(M, targets, feats16.astype(np.float64))
    M += cents16

    in_maps, host = [], []
    for k in range(n_cores):
        rids = order[k * BL:(k + 1) * BL]
        tcls = targets[rids]
        jlo = np.searchsorted(col_cls, tcls[0], "left")
        jhi = np.searchsorted(col_cls, tcls[-1], "right")
        assert jhi - jlo <= MW * P, f"core {k}: window {jhi - jlo} > {MW * P}"
        perm = np.concatenate([
            np.arange(jlo, jhi), np.arange(0, jlo), np.arange(jhi, J)
        ])

        Ap = np.zeros((JP, D), np.float16)
        Ap[:J] = A16[perm]
        up = np.full(JP, -1.0)
        up[:J] = col_cls[perm]
        biasv = np.full(JP, -200.0, np.float64)
        biasv[:J] = np.log(a_col[perm]) - SHIFT

        # scatter the column groups to processing slots: window tile m goes
        # to slot MSLOTS[m], the rest fill the remaining slots in order
        slot_of = np.empty(NT, np.int64)        # slot -> group index
        rest = [s for s in range(NT) if s not in MSLOTS]
        for m, s in enumerate(MSLOTS):
            slot_of[s] = m
        for g, s in enumerate(rest):
            slot_of[s] = MW + g
        cperm = (slot_of[:, None] * P + np.arange(P)[None, :]).reshape(-1)
        Ap = Ap[cperm]
        biasv = biasv[cperm]

        fq8 = feats16[rids].astype(F8NP)         # [BL, 128]
        A8 = Ap.astype(F8NP)                     # [JP, 128]

        im = {
            "bias": np.ascontiguousarray(biasv.reshape(NT, P).T.astype(np.float32)),
            "tIrow": tcls.astype(np.float16).reshape(1, BL),
            "tcol": np.ascontiguousarray(
                up[:MW * P].reshape(MW, P).T.astype(np.float16)
            ),
        }
        featq = np.ascontiguousarray(fq8.T)       # [128, BL]
        FS = BL // FCHUNK
        for c in range(FCHUNK):
            im[f"featq{c}"] = np.ascontiguousarray(featq[:, c * FS:(c + 1) * FS])
        fTq = A8.reshape(NT, P, P).transpose(2, 0, 1).reshape(P, NT * P)
        off = 0
        for ci, w in enumerate(WCHUNKS):
            im[f"fTq{ci}"] = np.ascontiguousarray(fTq[:, off * P:(off + w) * P])
            off += w
        in_maps.append(im)

        n_t = n[tcls].astype(np.float64)
        fqd = fq8.astype(np.float64)
        r2q = np.einsum("ij,ij->i", fqd, fqd)
        Eii = np.exp(SHIFT * r2q.astype(np.float32).astype(np.float64) - SHIFT)
        f16d = feats16[rids].astype(np.float64)
        r2n = np.einsum("ij,ij->i", f16d, f16d)
        numer = (SHIFT * (np.einsum("ij,ij->i", f16d, M[tcls]) - r2n)) / n_t - SHIFT
        host.append({"n_t": n_t, "Eii": Eii, "numer": numer})
    return in_maps, host


_NC_CACHE = {}


def _get_nc():
    if "nc" not in _NC_CACHE:
        _NC_CACHE["nc"] = build_nc()
    return _NC_CACHE["nc"]


def run(centers1, features, targets, trace=False):
    n_cores = 8
    nc = _get_nc()
    in_maps, host = prep_inputs(centers1, features, targets, n_cores)
    res = run_bass_kernel_spmd(nc, in_maps, list(range(n_cores)), trace=trace)
    mlps = []
    for k in range(n_cores):
        out = res.results[k]["out"].astype(np.float64).reshape(-1)  # [2*BL]
        S = out[0:BL]
        Pm = out[BL:2 * BL]
        h = host[k]
        Sfix = S + (Pm - h["Eii"]) / h["n_t"]
        mlps.append(h["numer"] - np.log(Sfix))
    loss = -np.mean(np.concatenate(mlps))
    return np.float32(loss), res


def kernel(centers1, features, targets):
    loss, _ = run(centers1, features, targets)
    return np.asarray(loss, dtype=np.float32)
